# revision 5
# baseline (speedup 1.0000x reference)
"""BinaryConv2d (3x3, SAME, NHWC) Trainium2 Bass kernel.

Strategy (v2: mixed-precision direct conv, cout-major):
  - Data-parallel over batch: 32 images -> 8 cores x 4 images. No collectives.
  - The 9-tap conv contraction (9 taps x 128 cin) is split by dtype:
      * 5 taps in bf16 (plain matmuls)
      * 4 taps in fp8 e4m3 packed as 2 DoubleRow matmuls (2 taps each,
        virtual K=256, ~2x MAC rate). Weights are +-1 (exact in fp8); only
        x pays e4m3 quantization (~2.65% elementwise), giving an end-to-end
        L2 rel err of ~1.77e-2 (verified offline vs the jax reference).
    Tap pairs are chosen so the two K-halves of each DoubleRow matmul sit
    112 fp8 elements apart in the same image plane (112 % 16 == 0, the
    LDW/AP alignment granule): pairs {(0,2),(1,0)} and {(1,2),(2,0)} of
    offsets o = (dh-1)*114 + dw. The rhs is a hand-built overlapping 3D AP
    [cin, 2 (step 112), 512 (step 1)].
  - cout-major output: out_psum [couthalf=128, 512 px] f32 (one full PSUM
    bank), lhsT = weights [cin, couthalf] stationary, rhs = image slices
    [cin, 512] moving (N=512 streams keep DoubleRow's 256-col LDWEIGHTS
    hidden under the 241ns matmuls). 7 matmuls accumulate per bank.
  - ScalarE activation(Identity, bias per-partition) unloads PSUM -> SBUF
    f32 and adds the (couthalf-indexed) bias in one pass; stores go out on
    the vector-engine HWDGE queue as [128, 512] f32 blocks into a
    channel-major DRAM tensor [img, half, couthalf, 12800 px]. The host
    transposes back to NHWC and strips pad columns.
  - Host prep: x cast to bf16 AND e4m3, both padded to width 114 with zero
    cols 0/113. The fp8 plane is uploaded as uint16-PAIRS (2 adjacent px
    packed) so the 2-byte HWDGE xbar transpose yields the byte-exact
    [cin, px] fp8 layout on SBUF.
  - Transposes paced behind the PE exactly like the proven baseline
    (explicit dep on an already-issued matmul, LEAD windows ahead).
"""

import numpy as np

N_CORES = 8
H = 112
W_DIM = 112
CIN = 128
COUT = 256
BATCH = 32
IMG_PER_CORE = BATCH // N_CORES

WP = 114          # padded row width (px)
NPX = 12800       # padded linear px per image (112*114=12768 + tail pad)
CHUNK = 512       # out px per psum bank
N_CHUNK = NPX // CHUNK  # 25
GUARD = 8         # bf16 tile guard rows (baseline-proven 32B-aligned)
GUARD8 = 16       # fp8 tile guard rows (16*114B dest offsets stay 32B-aligned)
WPH = WP // 2     # 57 uint16 pairs per row

# tap offsets o = (dh-1)*WP + dw  (input px offset for output px p)
BF16_TAPS = [(0, 0), (0, 1), (1, 1), (2, 1), (2, 2)]
FP8_PAIRS = [((0, 2), (1, 0)), ((1, 2), (2, 0))]  # each pair: o_B - o_A = 112


def _tap_off(dh, dw):
    return (dh - 1) * WP + dw


def _build_program(n_img):
    import bass_rust
    import concourse.bacc as bacc
    import concourse.mybir as mybir
    import concourse.tile as tile
    from concourse.bass_types import AP

    f32 = mybir.dt.float32
    bf16 = mybir.dt.bfloat16
    fp8 = mybir.dt.float8e4
    Ident = mybir.ActivationFunctionType.Identity
    DR = mybir.MatmulPerfMode.DoubleRow

    nc = bacc.Bacc(
        "TRN2", target_bir_lowering=False, debug=False, num_devices=N_CORES
    )
    x_d = nc.dram_tensor("x", [n_img, H, WP, CIN], bf16, kind="ExternalInput").ap()
    x8_d = nc.dram_tensor(
        "x8", [n_img, H, WPH, CIN], bf16, kind="ExternalInput"
    ).ap()  # byte payload: fp8 px-pairs packed little-endian
    wb_d = nc.dram_tensor("wb", [CIN, 5, 2, 128], bf16, kind="ExternalInput").ap()
    wp_d = nc.dram_tensor(
        "wp", [CIN, 2, 2, 2, 64], bf16, kind="ExternalInput"
    ).ap()  # byte payload: fp8 [cin, pair, slot, half, m] as uint16-pairs of m
    bv_d = nc.dram_tensor("bv", [128, 2], f32, kind="ExternalInput").ap()
    out_d = nc.dram_tensor(
        "out", [n_img, 2, 128, NPX], f32, kind="ExternalOutput"
    ).ap()

    tile_rows = GUARD + H + GUARD        # 128 rows in the bf16 image tile
    base = GUARD * WP                    # bf16 px offset of data row 0
    tile_rows8 = GUARD8 + H + GUARD8     # 144 rows in the fp8 image tile
    base8 = GUARD8 * WP                  # fp8 px offset of data row 0 (1824)

    with tile.TileContext(nc) as tc:
        with (
            tc.tile_pool(name="consts", bufs=1) as cpool,
            tc.tile_pool(name="ximg", bufs=n_img) as xpool,
            tc.tile_pool(name="psum", bufs=4, space="PSUM") as pspool,
            tc.tile_pool(name="outs", bufs=4) as opool,
        ):
            wb_t = cpool.tile([CIN, 5, 2, 128], bf16)
            nc.scalar.dma_start(out=wb_t[:], in_=wb_d[:])
            wp_t = cpool.tile([CIN, 2, 2, 2, 64], bf16)
            nc.scalar.dma_start(out=wp_t[:], in_=wp_d[:])
            bv_t = cpool.tile([128, 2], f32)
            nc.scalar.dma_start(out=bv_t[:], in_=bv_d[:])
            wp_v = wp_t[:].bitcast(fp8)  # [CIN, 2, 2, 2, 256] fp8

            # warm the sync queue: plain copy then micro-transpose so the
            # first real chunk's xbar path is already spun up
            warm0 = cpool.tile([16, CIN], bf16)
            nc.sync.dma_start(out=warm0[:], in_=x_d[0, 0, 0:16, :])
            warm = cpool.tile([CIN, 16], bf16)
            nc.sync.dma_start(out=warm[:], in_=x_d[0, 0, 0:16, :], transpose=True)

            xb = [None] * n_img
            x8 = [None] * n_img
            for img in range(n_img):
                bt = xpool.tile([CIN, tile_rows * WP], bf16, tag="xb")
                xb[img] = bt
                nc.vector.memset(bt[:, 0:base], 0.0)
                nc.vector.memset(bt[:, base + H * WP :], 0.0)
                ft = xpool.tile([CIN, tile_rows8 * WPH], bf16, tag="x8")
                x8[img] = ft
                nc.vector.memset(ft[:, 0 : base8 // 2], 0.0)
                nc.vector.memset(ft[:, (base8 + H * WP) // 2 :], 0.0)

            # transpose work list in global consumption order, paced behind
            # the PE via explicit deps (baseline-proven).
            LEAD = 5  # 512-px chunks of lead (= 2560 px)
            chunks = []
            for img in range(n_img):
                if img == 0:
                    bsizes = [8, 8] + [16] * ((H - 16) // 16)
                else:
                    bsizes = [16] * (H // 16)
                items = []
                r0 = 0
                for sz in bsizes:
                    def mkb(img=img, r0=r0, sz=sz):
                        def issue():
                            return nc.sync.dma_start(
                                out=xb[img][
                                    :, (GUARD + r0) * WP : (GUARD + r0 + sz) * WP
                                ],
                                in_=x_d[img, r0 : r0 + sz].rearrange(
                                    "a b c -> (a b) c"
                                ),
                                transpose=True,
                            )
                        return issue
                    items.append((r0, 0, mkb()))
                    r0 += sz
                r0 = 0
                for sz in [16] * (H // 16):
                    def mkf(img=img, r0=r0, sz=sz):
                        def issue():
                            return nc.sync.dma_start(
                                out=x8[img][
                                    :,
                                    (GUARD8 + r0) * WPH : (GUARD8 + r0 + sz) * WPH,
                                ],
                                in_=x8_d[img, r0 : r0 + sz].rearrange(
                                    "a b c -> (a b) c"
                                ),
                                transpose=True,
                            )
                        return issue
                    items.append((r0, 1, mkf()))
                    r0 += sz
                items.sort(key=lambda it: (it[0], it[1]))
                for r0, _, fn in items:
                    trigger = max(0, (img * NPX + r0 * WP) // CHUNK - LEAD)
                    chunks.append([trigger, fn])
            next_chunk = 0
            while next_chunk < len(chunks) and chunks[next_chunk][0] == 0:
                chunks[next_chunk][1]()
                next_chunk += 1

            def dr_rhs(img, start_px):
                s = x8[img][:].bitcast(fp8)[:, start_px : start_px + CHUNK]
                return AP(s.tensor, s.offset, [list(s.ap[0]), [112, 2], [1, CHUNK]])

            last_mm = None
            for img in range(n_img):
                for wg in range(N_CHUNK):
                    gw = img * N_CHUNK + wg
                    while next_chunk < len(chunks) and chunks[next_chunk][0] <= gw:
                        tr = chunks[next_chunk][1]()
                        bass_rust.add_dep_helper(
                            tr.ins,
                            last_mm.ins,
                            sync=True,
                            reason="pace transposes behind the PE",
                        )
                        next_chunk += 1
                    p0 = wg * CHUNK
                    for h in range(2):
                        ps = pspool.tile([128, CHUNK], f32, tag="ps")
                        for k, (dh, dw) in enumerate(BF16_TAPS):
                            o = base + p0 + _tap_off(dh, dw)
                            last_mm = nc.tensor.matmul(
                                ps[:],
                                wb_t[:, k, h, :],
                                xb[img][:, o : o + CHUNK],
                                start=(k == 0),
                                stop=False,
                            )
                        for pr, (tA, _tB) in enumerate(FP8_PAIRS):
                            oA = base8 + p0 + _tap_off(*tA)
                            last_mm = nc.tensor.matmul(
                                ps[:],
                                wp_v[:, pr, :, h, :],
                                dr_rhs(img, oA),
                                start=False,
                                stop=(pr == 1),
                                perf_mode=DR,
                            )
                        st = opool.tile([128, CHUNK], f32, tag="st")
                        nc.scalar.activation(
                            st[:], ps[:], Ident, bias=bv_t[:, h : h + 1], scale=1.0
                        )
                        nc.scalar.dma_start(
                            out=out_d[img, h, :, p0 : p0 + CHUNK], in_=st[:]
                        )

    nc.compile()
    return nc


_cached_nc = None


def _get_program():
    global _cached_nc
    if _cached_nc is None:
        _cached_nc = _build_program(IMG_PER_CORE)
    return _cached_nc


def _prep_inputs(x, W, b):
    import ml_dtypes

    bf16 = ml_dtypes.bfloat16
    e4m3 = ml_dtypes.float8_e4m3

    wq = np.sign(W.astype(np.float32)).astype(np.float32)  # [3,3,cin,cout]
    # bf16 taps: wb[cin, 5, 2, 128]
    wb = np.empty((CIN, 5, 2, 128), dtype=bf16)
    for k, (dh, dw) in enumerate(BF16_TAPS):
        wb[:, k, 0, :] = wq[dh, dw, :, :128].astype(bf16)
        wb[:, k, 1, :] = wq[dh, dw, :, 128:].astype(bf16)
    # fp8 pairs: wp[cin, pair, slot, half, 128] fp8, uploaded as u16 pairs
    wp8 = np.empty((CIN, 2, 2, 2, 128), dtype=e4m3)
    for pr, (tA, tB) in enumerate(FP8_PAIRS):
        for s, (dh, dw) in enumerate((tA, tB)):
            wp8[:, pr, s, 0, :] = wq[dh, dw, :, :128].astype(e4m3)
            wp8[:, pr, s, 1, :] = wq[dh, dw, :, 128:].astype(e4m3)
    wpu = wp8.view(np.uint8).astype(np.uint16)
    wp16 = (wpu[..., 0::2] | (wpu[..., 1::2] << 8)).view(bf16)  # [...,128]->u16 pairs

    bv = np.ascontiguousarray(
        b.astype(np.float32).reshape(2, 128).T
    )  # [128 m, 2 half]

    xf = x.astype(np.float32)
    xpad = np.zeros((BATCH, H, WP, CIN), dtype=np.float32)
    xpad[:, :, 1 : W_DIM + 1, :] = xf
    xb = xpad.astype(bf16)
    x8 = xpad.astype(e4m3)
    # pack adjacent px pairs into u16 (little-endian: low byte = even px)
    x8u = x8.view(np.uint8).astype(np.uint16)
    x8p = (x8u[:, :, 0::2, :] | (x8u[:, :, 1::2, :] << 8)).view(bf16)
    # [BATCH, H, 57, CIN]

    in_maps = []
    for c in range(N_CORES):
        sl = slice(c * IMG_PER_CORE, (c + 1) * IMG_PER_CORE)
        in_maps.append(
            {
                "x": np.ascontiguousarray(xb[sl]),
                "x8": np.ascontiguousarray(x8p[sl]),
                "wb": wb,
                "wp": wp16,
                "bv": bv,
            }
        )
    return in_maps


def run(x, W, b, trace=False, tmpdir=None):
    from concourse import bass_utils

    if trace:
        # the agent image's antenv lacks axon_hooks; wire the NTFF profile
        # hook up manually so trace=True yields exec_time_ns + pftrace
        import sys, types

        if "antenv.axon_hooks" not in sys.modules:
            import antenv
            from trn_agent_boot.trn_boot import _ntff_profile_via_ctypes

            mod = types.ModuleType("antenv.axon_hooks")
            _hook = _ntff_profile_via_ctypes("/opt/axon/libaxon_pjrt.so")
            mod.get_axon_ntff_profile_hook = lambda: _hook
            sys.modules["antenv.axon_hooks"] = mod
            antenv.axon_hooks = mod

    nc = _get_program()
    in_maps = _prep_inputs(x, W, b)
    res = bass_utils.run_bass_kernel_spmd(
        nc, in_maps, list(range(N_CORES)), trace=trace, tmpdir=tmpdir
    )
    # device output is channel-major [n_img, 2, 128, 12800]; host restores
    # NHWC and strips the pad cols (c=112,113) and the tail
    outs = []
    for i in range(N_CORES):
        o = res.results[i]["out"]  # [n_img, 2, 128, NPX]
        o = o.reshape(IMG_PER_CORE, COUT, NPX)[:, :, : H * WP]
        o = o.reshape(IMG_PER_CORE, COUT, H, WP)[:, :, :, :W_DIM]
        outs.append(o.transpose(0, 2, 3, 1))
    out = np.ascontiguousarray(np.concatenate(outs, axis=0), dtype=np.float32)
    return out, res


def kernel(x, W, b):
    out, _ = run(x, W, b, trace=False)
    return out


# revision 8
# speedup vs baseline: 1.0060x; 1.0060x over previous
"""BinaryConv2d (3x3, SAME, NHWC) Trainium2 Bass kernel.

Strategy (v2: mixed-precision direct conv, cout-major):
  - Data-parallel over batch: 32 images -> 8 cores x 4 images. No collectives.
  - The 9-tap conv contraction (9 taps x 128 cin) is split by dtype:
      * 5 taps in bf16 (plain matmuls)
      * 4 taps in fp8 e4m3 packed as 2 DoubleRow matmuls (2 taps each,
        virtual K=256, ~2x MAC rate). Weights are +-1 (exact in fp8); only
        x pays e4m3 quantization (~2.65% elementwise), giving an end-to-end
        L2 rel err of ~1.77e-2 (verified offline vs the jax reference).
    Tap pairs are chosen so the two K-halves of each DoubleRow matmul sit
    112 fp8 elements apart in the same image plane (112 % 16 == 0, the
    LDW/AP alignment granule): pairs {(0,2),(1,0)} and {(1,2),(2,0)} of
    offsets o = (dh-1)*114 + dw. The rhs is a hand-built overlapping 3D AP
    [cin, 2 (step 112), 512 (step 1)].
  - cout-major output: out_psum [couthalf=128, 512 px] f32 (one full PSUM
    bank), lhsT = weights [cin, couthalf] stationary, rhs = image slices
    [cin, 512] moving (N=512 streams keep DoubleRow's 256-col LDWEIGHTS
    hidden under the 241ns matmuls). 7 matmuls accumulate per bank.
  - Output: each finished PSUM bank [128, 512] f32 is unloaded to
    SBUF by a DVE tensor_copy (the DVE is otherwise idle; ~660ns) and
    DMA'd out on the Scalar HWDGE queue into a channel-major tensor
    [img, half, couthalf, 12800 px]; the host transposes back to NHWC,
    strips pad columns, and adds the bias (a [256] broadcast; the device
    conv is bias-free). This keeps the Scalar HWDGE queue down to one
    600ns trigger per chunk-half -- the v2 ACT+staging pipeline serialized
    ACTIVATE + trigger + staging-free waits on one queue and starved the
    PE ~190us.
  - Host prep: x cast to bf16 AND e4m3, both padded to width 114 with zero
    cols 0/113. The fp8 plane is uploaded as uint16-PAIRS (2 adjacent px
    packed) so the 2-byte HWDGE xbar transpose yields the byte-exact
    [cin, px] fp8 layout on SBUF.
  - Transposes paced behind the PE exactly like the proven baseline
    (explicit dep on an already-issued matmul, LEAD windows ahead).
"""

import numpy as np

N_CORES = 8
H = 112
W_DIM = 112
CIN = 128
COUT = 256
BATCH = 32
IMG_PER_CORE = BATCH // N_CORES

WP = 114          # padded row width (px)
NPX = 12800       # padded linear px per image (112*114=12768 + tail pad)
CHUNK = 512       # out px per psum bank
N_CHUNK = NPX // CHUNK  # 25
GUARD = 8         # bf16 tile guard rows (baseline-proven 32B-aligned)
GUARD8 = 16       # fp8 tile guard rows (16*114B dest offsets stay 32B-aligned)
WPH = WP // 2     # 57 uint16 pairs per row

# tap offsets o = (dh-1)*WP + dw  (input px offset for output px p)
BF16_TAPS = [(0, 0), (0, 1), (1, 1), (2, 1), (2, 2)]
FP8_PAIRS = [((0, 2), (1, 0)), ((1, 2), (2, 0))]  # each pair: o_B - o_A = 112


def _tap_off(dh, dw):
    return (dh - 1) * WP + dw


def _build_program(n_img):
    import bass_rust
    import concourse.bacc as bacc
    import concourse.mybir as mybir
    import concourse.tile as tile
    from concourse.bass_types import AP

    f32 = mybir.dt.float32
    bf16 = mybir.dt.bfloat16
    fp8 = mybir.dt.float8e4
    DR = mybir.MatmulPerfMode.DoubleRow

    nc = bacc.Bacc(
        "TRN2", target_bir_lowering=False, debug=False, num_devices=N_CORES
    )
    x_d = nc.dram_tensor("x", [n_img, H, WP, CIN], bf16, kind="ExternalInput").ap()
    x8_d = nc.dram_tensor(
        "x8", [n_img, H, WPH, CIN], bf16, kind="ExternalInput"
    ).ap()  # byte payload: fp8 px-pairs packed little-endian
    wb_d = nc.dram_tensor("wb", [CIN, 5, 2, 128], bf16, kind="ExternalInput").ap()
    wp_d = nc.dram_tensor(
        "wp", [CIN, 2, 2, 2, 64], bf16, kind="ExternalInput"
    ).ap()  # byte payload: fp8 [cin, pair, slot, half, m] as uint16-pairs of m
    out_d = nc.dram_tensor(
        "out", [n_img, 2, 128, NPX], f32, kind="ExternalOutput"
    ).ap()

    tile_rows = GUARD + H + GUARD        # 128 rows in the bf16 image tile
    base = GUARD * WP                    # bf16 px offset of data row 0
    tile_rows8 = GUARD8 + H + GUARD8     # 144 rows in the fp8 image tile
    base8 = GUARD8 * WP                  # fp8 px offset of data row 0 (1824)

    with tile.TileContext(nc) as tc:
        with (
            tc.tile_pool(name="consts", bufs=1) as cpool,
            tc.tile_pool(name="ximg", bufs=n_img) as xpool,
            tc.tile_pool(name="psum", bufs=6, space="PSUM") as pspool,
            tc.tile_pool(name="outs", bufs=4) as opool,
        ):
            wb_t = cpool.tile([CIN, 5, 2, 128], bf16)
            nc.scalar.dma_start(out=wb_t[:], in_=wb_d[:])
            wp_t = cpool.tile([CIN, 2, 2, 2, 64], bf16)
            nc.scalar.dma_start(out=wp_t[:], in_=wp_d[:])
            wp_v = wp_t[:].bitcast(fp8)  # [CIN, 2, 2, 2, 256] fp8

            # warm the sync queue: plain copy then micro-transpose so the
            # first real chunk's xbar path is already spun up
            warm0 = cpool.tile([16, CIN], bf16)
            nc.sync.dma_start(out=warm0[:], in_=x_d[0, 0, 0:16, :])
            warm = cpool.tile([CIN, 16], bf16)
            nc.sync.dma_start(out=warm[:], in_=x_d[0, 0, 0:16, :], transpose=True)

            xb = [None] * n_img
            x8 = [None] * n_img
            for img in range(n_img):
                bt = xpool.tile([CIN, tile_rows * WP], bf16, tag="xb")
                xb[img] = bt
                nc.vector.memset(bt[:, 0:base], 0.0)
                nc.vector.memset(bt[:, base + H * WP :], 0.0)
                ft = xpool.tile([CIN, tile_rows8 * WPH], bf16, tag="x8")
                x8[img] = ft
                nc.vector.memset(ft[:, 0 : base8 // 2], 0.0)
                nc.vector.memset(ft[:, (base8 + H * WP) // 2 :], 0.0)

            # transpose work list in global consumption order, paced behind
            # the PE via explicit deps (baseline-proven).
            LEAD = 5  # 512-px chunks of lead (= 2560 px)
            chunks = []
            for img in range(n_img):
                if img == 0:
                    bsizes = [8, 8] + [16] * ((H - 16) // 16)
                else:
                    bsizes = [16] * (H // 16)
                items = []
                r0 = 0
                for sz in bsizes:
                    def mkb(img=img, r0=r0, sz=sz):
                        def issue():
                            return nc.sync.dma_start(
                                out=xb[img][
                                    :, (GUARD + r0) * WP : (GUARD + r0 + sz) * WP
                                ],
                                in_=x_d[img, r0 : r0 + sz].rearrange(
                                    "a b c -> (a b) c"
                                ),
                                transpose=True,
                            )
                        return issue
                    items.append((r0, 0, mkb()))
                    r0 += sz
                r0 = 0
                for sz in [16] * (H // 16):
                    def mkf(img=img, r0=r0, sz=sz):
                        def issue():
                            return nc.sync.dma_start(
                                out=x8[img][
                                    :,
                                    (GUARD8 + r0) * WPH : (GUARD8 + r0 + sz) * WPH,
                                ],
                                in_=x8_d[img, r0 : r0 + sz].rearrange(
                                    "a b c -> (a b) c"
                                ),
                                transpose=True,
                            )
                        return issue
                    items.append((r0, 1, mkf()))
                    r0 += sz
                items.sort(key=lambda it: (it[0], it[1]))
                for r0, _, fn in items:
                    trigger = max(0, (img * NPX + r0 * WP) // CHUNK - LEAD)
                    chunks.append([trigger, fn])
            next_chunk = 0
            while next_chunk < len(chunks) and chunks[next_chunk][0] == 0:
                chunks[next_chunk][1]()
                next_chunk += 1

            def dr_rhs(img, start_px):
                s = x8[img][:].bitcast(fp8)[:, start_px : start_px + CHUNK]
                return AP(s.tensor, s.offset, [list(s.ap[0]), [112, 2], [1, CHUNK]])

            last_mm = None
            for img in range(n_img):
                for wg in range(N_CHUNK):
                    gw = img * N_CHUNK + wg
                    while next_chunk < len(chunks) and chunks[next_chunk][0] <= gw:
                        tr = chunks[next_chunk][1]()
                        bass_rust.add_dep_helper(
                            tr.ins,
                            last_mm.ins,
                            sync=True,
                            reason="pace transposes behind the PE",
                        )
                        next_chunk += 1
                    p0 = wg * CHUNK
                    for h in range(2):
                        ps = pspool.tile([128, CHUNK], f32, tag="ps")
                        for k, (dh, dw) in enumerate(BF16_TAPS):
                            o = base + p0 + _tap_off(dh, dw)
                            last_mm = nc.tensor.matmul(
                                ps[:],
                                wb_t[:, k, h, :],
                                xb[img][:, o : o + CHUNK],
                                start=(k == 0),
                                stop=False,
                            )
                        for pr, (tA, _tB) in enumerate(FP8_PAIRS):
                            oA = base8 + p0 + _tap_off(*tA)
                            last_mm = nc.tensor.matmul(
                                ps[:],
                                wp_v[:, pr, :, h, :],
                                dr_rhs(img, oA),
                                start=False,
                                stop=(pr == 1),
                                perf_mode=DR,
                            )
                        st = opool.tile([128, CHUNK], f32, tag="st")
                        nc.vector.tensor_copy(st[:], ps[:])
                        nc.scalar.dma_start(
                            out=out_d[img, h, :, p0 : p0 + CHUNK], in_=st[:]
                        )

    nc.compile()
    return nc


_cached_nc = None


def _get_program():
    global _cached_nc
    if _cached_nc is None:
        _cached_nc = _build_program(IMG_PER_CORE)
    return _cached_nc


def _prep_inputs(x, W, b):
    import ml_dtypes

    bf16 = ml_dtypes.bfloat16
    e4m3 = ml_dtypes.float8_e4m3

    wq = np.sign(W.astype(np.float32)).astype(np.float32)  # [3,3,cin,cout]
    # bf16 taps: wb[cin, 5, 2, 128]
    wb = np.empty((CIN, 5, 2, 128), dtype=bf16)
    for k, (dh, dw) in enumerate(BF16_TAPS):
        wb[:, k, 0, :] = wq[dh, dw, :, :128].astype(bf16)
        wb[:, k, 1, :] = wq[dh, dw, :, 128:].astype(bf16)
    # fp8 pairs: wp[cin, pair, slot, half, 128] fp8, uploaded as u16 pairs
    wp8 = np.empty((CIN, 2, 2, 2, 128), dtype=e4m3)
    for pr, (tA, tB) in enumerate(FP8_PAIRS):
        for s, (dh, dw) in enumerate((tA, tB)):
            wp8[:, pr, s, 0, :] = wq[dh, dw, :, :128].astype(e4m3)
            wp8[:, pr, s, 1, :] = wq[dh, dw, :, 128:].astype(e4m3)
    wpu = wp8.view(np.uint8).astype(np.uint16)
    wp16 = (wpu[..., 0::2] | (wpu[..., 1::2] << 8)).view(bf16)  # [...,128]->u16 pairs

    xf = x.astype(np.float32)
    xpad = np.zeros((BATCH, H, WP, CIN), dtype=np.float32)
    xpad[:, :, 1 : W_DIM + 1, :] = xf
    xb = xpad.astype(bf16)
    x8 = xpad.astype(e4m3)
    # pack adjacent px pairs into u16 (little-endian: low byte = even px)
    x8u = x8.view(np.uint8).astype(np.uint16)
    x8p = (x8u[:, :, 0::2, :] | (x8u[:, :, 1::2, :] << 8)).view(bf16)
    # [BATCH, H, 57, CIN]

    in_maps = []
    for c in range(N_CORES):
        sl = slice(c * IMG_PER_CORE, (c + 1) * IMG_PER_CORE)
        in_maps.append(
            {
                "x": np.ascontiguousarray(xb[sl]),
                "x8": np.ascontiguousarray(x8p[sl]),
                "wb": wb,
                "wp": wp16,
            }
        )
    return in_maps


def run(x, W, b, trace=False, tmpdir=None):
    from concourse import bass_utils

    if trace:
        # the agent image's antenv lacks axon_hooks; wire the NTFF profile
        # hook up manually so trace=True yields exec_time_ns + pftrace
        import sys, types

        if "antenv.axon_hooks" not in sys.modules:
            import antenv
            from trn_agent_boot.trn_boot import _ntff_profile_via_ctypes

            mod = types.ModuleType("antenv.axon_hooks")
            _hook = _ntff_profile_via_ctypes("/opt/axon/libaxon_pjrt.so")
            mod.get_axon_ntff_profile_hook = lambda: _hook
            sys.modules["antenv.axon_hooks"] = mod
            antenv.axon_hooks = mod

    nc = _get_program()
    in_maps = _prep_inputs(x, W, b)
    res = bass_utils.run_bass_kernel_spmd(
        nc, in_maps, list(range(N_CORES)), trace=trace, tmpdir=tmpdir
    )
    # device output is channel-major [n_img, 2, 128, 12800]; host restores
    # NHWC and strips the pad cols (c=112,113) and the tail
    outs = []
    for i in range(N_CORES):
        o = res.results[i]["out"]  # [n_img, 2, 128, NPX]
        o = o.reshape(IMG_PER_CORE, COUT, NPX)[:, :, : H * WP]
        o = o.reshape(IMG_PER_CORE, COUT, H, WP)[:, :, :, :W_DIM]
        outs.append(o.transpose(0, 2, 3, 1))
    out = np.ascontiguousarray(np.concatenate(outs, axis=0), dtype=np.float32)
    out += b.astype(np.float32)  # bias folded on host (device conv is bias-free)
    return out, res


def kernel(x, W, b):
    out, _ = run(x, W, b, trace=False)
    return out
